# revision 26
# baseline (speedup 1.0000x reference)
"""RGCN graph-scoring kernel for Trainium2 (8 NeuronCores, one graph per core).

Math (per graph):
  out = relu(x @ root + bias + sum_r mean_r @ W_r);  scores = out @ lin + linb
  mean_r[n] = mean of x[src_e] over edges e with dst_e == n, type_e == r.

Device strategy per core (v7 -- serial phases, each tightened):
  HW note: while Q7 dma_gather descriptor generation runs, HWDGE DMA
  dispatch (writes) freezes chip-wide, so phase 1 and gather desc-gen are
  kept strictly disjoint in time and each is made as fast as possible.

  1. Phase 1: xw[src*8 + r_local] = (x @ W_r)[src] on PE in bf16, staged
     to DRAM. Scoped pools give the PSUM staging 4 double-bank buffers
     (all 8 banks, released before phase 2); the PSUM->SBUF cast copy is
     split across ACT and DVE per chunk; writes alternate the ACT and
     sync HWDGE queues.
  2. Gathers: 16 plain dma_gathers (4 dst-tiles each, two r halves so
     indices fit int16), round-robined over the 4 SWDGE queues -- the Q7
     pairs desc-gen concurrently and each gather's transfer auto-fires,
     overlapping the next gather's desc-gen. A tiny warm-up gather at
     program start forces the Q7 ucode library load while nothing is
     in flight.
  3. Per dst tile: PSUM acc[c', m] seeded by the root matmul, then one
     bf16 matmul per 128-edge chunk: acc += z_chunk^T @ OHa with
     OHa[e, m] = alpha_e * (dstloc_e == m) built by one fused DVE
     tensor_scalar (is_equal then mult). alpha_e = 1/cnt(type_e, dst_e);
     pad edges have alpha = 0 and index 0. relu+bias on ACT, head
     matmul, ACT copy into a resident bf16 scores row; ONE final DMA out
     (no per-tile writes that could land inside desc-gen windows).
     linb is added on the host, which also casts scores back to f32.
"""

import sys

for _p in ("/opt/trn_rl_repo", "/root/.axon_site/_ro/trn_rl_repo"):
    if _p not in sys.path:
        sys.path.insert(0, _p)

import numpy as np
import ml_dtypes

import concourse.bacc as bacc
import concourse.mybir as mybir
from concourse.tile import TileContext
from concourse.bass_utils import run_bass_kernel_spmd

BF16 = ml_dtypes.bfloat16
P = 128
B, N, C, R, E = 8, 4096, 128, 16, 65536
NT = N // P  # 32 node tiles
NH = 2  # r halves
RH = R // NH  # 8 relations per half
TG = 4  # dst tiles per merged gather
NG = NT // TG  # 8 tile groups
NBINS = NT * NH  # logical (tile, half) sub-bins
DEF_CAP = 1152  # per-(tile, half) edge capacity; mean 1024, +4 sigma
NQ = 4  # SWDGE queues

_prog_cache = {}


def build_program(cap):
    """Build + compile the SPMD Bass program for sub-bin capacity `cap`."""
    assert cap % P == 0
    nch = cap // P  # chunks per sub-bin
    mcap = TG * cap  # merged gather capacity
    etot = NBINS * cap  # padded edge count
    nchunks = etot // P

    nc = bacc.Bacc("TRN2", num_swdge_queues=NQ)
    f32 = mybir.dt.float32
    bf16 = mybir.dt.bfloat16

    xT = nc.dram_tensor("xT", [P, N], bf16, kind="ExternalInput")
    wcat = nc.dram_tensor("wcat", [P, R * C], bf16, kind="ExternalInput")
    root = nc.dram_tensor("root", [P, C], bf16, kind="ExternalInput")
    bias = nc.dram_tensor("bias", [P, 1], f32, kind="ExternalInput")
    lin = nc.dram_tensor("lin", [P, 1], bf16, kind="ExternalInput")
    iota = nc.dram_tensor("iota", [P, P], bf16, kind="ExternalInput")
    gidx = nc.dram_tensor("gidx", [P, etot // 16], mybir.dt.int16, kind="ExternalInput")
    dstloc = nc.dram_tensor("dstloc", [P, nchunks], f32, kind="ExternalInput")
    alpha = nc.dram_tensor("alpha", [P, nchunks], f32, kind="ExternalInput")
    scores = nc.dram_tensor("scores", [1, N], bf16, kind="ExternalOutput")

    with TileContext(nc) as tc:
        with (
            tc.tile_pool(name="const", bufs=1) as cpool,
            tc.tile_pool(name="oh", bufs=8) as ohpool,
            tc.tile_pool(name="post", bufs=4) as ppool,
            tc.tile_pool(name="dram", bufs=1, space="DRAM") as dpool,
        ):
            # ---- resident inputs ----
            xT_t = cpool.tile([P, N], bf16)
            nc.sync.dma_start(out=xT_t[:], in_=xT[:])
            root_t = cpool.tile([P, C], bf16)
            nc.sync.dma_start(out=root_t[:], in_=root[:])
            bias_t = cpool.tile([P, 1], f32)
            nc.sync.dma_start(out=bias_t[:], in_=bias[:])
            lin_t = cpool.tile([P, 1], bf16)
            nc.sync.dma_start(out=lin_t[:], in_=lin[:])
            iota_t = cpool.tile([P, P], bf16)
            nc.sync.dma_start(out=iota_t[:], in_=iota[:])
            idx_t = cpool.tile([P, etot // 16], mybir.dt.int16)
            nc.sync.dma_start(out=idx_t[:], in_=gidx[:])
            dst_t = cpool.tile([P, nchunks], f32)
            nc.sync.dma_start(out=dst_t[:], in_=dstloc[:])
            alpha_t = cpool.tile([P, nchunks], f32)
            nc.sync.dma_start(out=alpha_t[:], in_=alpha[:])
            # all gathered edge rows; column block cidx*128 = global chunk cidx
            zbig = cpool.tile([P, etot], bf16)
            scores_t = cpool.tile([1, N], bf16)

            # DRAM scratch: per-half transformed features, row = src*8+r_local
            xw = [
                dpool.tile([N * RH, C], bf16, name=f"xw{h}", tag=f"xw{h}")
                for h in range(NH)
            ]

            # Warm up the Q7 gather ucode library before phase 1: the first
            # gather-family instruction triggers a LOAD_LIB that quiesces all
            # outstanding DMAs at its stream position.
            zwarm = cpool.tile([P, 1, P], bf16)
            nc.gpsimd.dma_gather(
                zwarm[:],
                xw[0][:],
                idx_t[:, 0:1],
                16,
                16,
                C,
                single_packet=False,
                queue_num=0,
            )

            # ---- phase 1: xw = x @ W_r (bf16), both halves ----
            with (
                tc.tile_pool(name="ph1", bufs=1) as ph1pool,
                tc.tile_pool(name="stage", bufs=6) as spool,
                tc.tile_pool(name="pxw", bufs=4, space="PSUM") as pxw_pool,
            ):
                wcat_t = ph1pool.tile([P, R * C], bf16)
                nc.sync.dma_start(out=wcat_t[:], in_=wcat[:])
                for h in range(NH):
                    for nchunk in range(NT):
                        pxw = pxw_pool.tile([P, RH * C], f32, space="PSUM")
                        for g in range(2):
                            nc.tensor.matmul(
                                out=pxw[:, g * 512 : (g + 1) * 512],
                                lhsT=xT_t[:, nchunk * P : (nchunk + 1) * P],
                                rhs=wcat_t[
                                    :,
                                    h * 1024 + g * 512 : h * 1024 + (g + 1) * 512,
                                ],
                                start=True,
                                stop=True,
                            )
                        stg = spool.tile([P, RH * C], bf16, tag="stage")
                        # split the PSUM->SBUF cast across both engines
                        nc.scalar.activation(
                            out=stg[:, :512],
                            in_=pxw[:, :512],
                            func=mybir.ActivationFunctionType.Copy,
                        )
                        nc.vector.tensor_scalar(
                            out=stg[:, 512:],
                            in0=pxw[:, 512:],
                            scalar1=0.0,
                            scalar2=None,
                            op0=mybir.AluOpType.add,
                        )
                        # stage [p, (rl, c')] -> xw[h] rows (nchunk*128+p)*8+rl
                        dst_view = xw[h][:].rearrange(
                            "(nt p rl) c -> nt p rl c", nt=NT, p=P, rl=RH
                        )[nchunk]
                        wr_eng = nc.scalar if nchunk % 2 == 0 else nc.sync
                        wr_eng.dma_start(
                            out=dst_view,
                            in_=stg[:].rearrange("p (rl c) -> p rl c", rl=RH),
                        )

            # ---- gathers: plain, 4-queue round robin; transfers auto-fire
            # per gather and overlap the next gather's desc-gen ----
            for g in range(NG):
                for h in range(NH):
                    mb = g * NH + h
                    z_view = zbig[:, mb * mcap : (mb + 1) * mcap].rearrange(
                        "p (ch c) -> p ch c", ch=TG * nch
                    )
                    nc.gpsimd.dma_gather(
                        z_view,
                        xw[h][:],
                        idx_t[:, mb * (mcap // 16) : (mb + 1) * (mcap // 16)],
                        mcap,
                        mcap,
                        C,
                        single_packet=False,
                        queue_num=mb % NQ,
                    )

            # ---- phase 2: aggregate per dst tile ----
            with (
                tc.tile_pool(name="pacc", bufs=3, space="PSUM") as pacc_pool,
                tc.tile_pool(name="plin", bufs=2, space="PSUM") as plin_pool,
            ):
                for t in range(NT):
                    acc = pacc_pool.tile([P, P], f32, space="PSUM", tag="acc")
                    # root term seeds the accumulator
                    nc.tensor.matmul(
                        out=acc[:],
                        lhsT=root_t[:],
                        rhs=xT_t[:, t * P : (t + 1) * P],
                        start=True,
                        stop=False,
                    )
                    for h in range(NH):
                        c0 = ((t // TG) * NH + h) * TG * nch + (t % TG) * nch
                        for c in range(nch):
                            cidx = c0 + c
                            oh = ohpool.tile([P, P], bf16, tag="oh")
                            nc.vector.tensor_scalar(
                                out=oh[:],
                                in0=iota_t[:],
                                scalar1=dst_t[:, cidx : cidx + 1],
                                scalar2=alpha_t[:, cidx : cidx + 1],
                                op0=mybir.AluOpType.is_equal,
                                op1=mybir.AluOpType.mult,
                            )
                            nc.tensor.matmul(
                                out=acc[:],
                                lhsT=zbig[:, cidx * P : (cidx + 1) * P],
                                rhs=oh[:],
                                start=False,
                                stop=(h == NH - 1 and c == nch - 1),
                            )
                    # relu(acc + bias) -> SBUF bf16
                    relu_t = ppool.tile([P, P], bf16, tag="relu")
                    nc.scalar.activation(
                        out=relu_t[:],
                        in_=acc[:],
                        func=mybir.ActivationFunctionType.Relu,
                        bias=bias_t[:, :1],
                    )
                    plin = plin_pool.tile([1, P], f32, space="PSUM", tag="plin")
                    nc.tensor.matmul(
                        out=plin[:],
                        lhsT=lin_t[:],
                        rhs=relu_t[:],
                        start=True,
                        stop=True,
                    )
                    nc.scalar.activation(
                        out=scores_t[:, t * P : (t + 1) * P],
                        in_=plin[:],
                        func=mybir.ActivationFunctionType.Copy,
                    )
            nc.sync.dma_start(out=scores[:], in_=scores_t[:])

    nc.compile()
    return nc


def _pack_core_inputs(x, ei, et, rel_w, root_w, rgcn_b, lin_w, lin_b, cap):
    """Host-side prep for one graph: sort/pad edges, pack device layouts."""
    src = ei[0].astype(np.int64)
    dst = ei[1].astype(np.int64)
    et = et.astype(np.int64)

    cnt = np.bincount(et * N + dst, minlength=R * N).astype(np.float32)
    alpha_e = 1.0 / cnt[et * N + dst]  # every edge's (r, dst) has cnt >= 1

    t_e = dst >> 7
    h_e = et >> 3
    rl_e = et & 7
    # sub-bin order: (tile group, half, tile within group)
    binid = ((t_e // TG) * NH + h_e) * TG + (t_e % TG)
    order = np.argsort(binid, kind="stable")

    etot = NBINS * cap
    g = np.zeros(etot, np.int16)
    dl = np.full(etot, 999.0, np.float32)
    al = np.zeros(etot, np.float32)

    counts = np.bincount(binid, minlength=NBINS)
    if counts.max() > cap:
        raise OverflowError(int(counts.max()))
    starts = np.zeros(NBINS, np.int64)
    starts[1:] = np.cumsum(counts)[:-1]
    # position of each (sorted) edge inside the padded sub-bin layout
    pos = np.arange(E) - starts[binid[order]] + np.arange(NBINS)[binid[order]] * cap
    g[pos] = (src[order] * 8 + rl_e[order]).astype(np.int16)
    dl[pos] = (dst[order] & 127).astype(np.float32)
    al[pos] = alpha_e[order].astype(np.float32)

    gidx = np.tile(g.reshape(-1, 16).T, (8, 1)).copy()  # [128, etot/16]
    dstloc = dl.reshape(-1, P).T.copy()  # [128, nchunks]
    alpha = al.reshape(-1, P).T.copy()

    return {
        "xT": np.ascontiguousarray(x.T).astype(BF16),
        "wcat": np.ascontiguousarray(
            rel_w.transpose(1, 0, 2).reshape(C, R * C)
        ).astype(BF16),
        "root": np.ascontiguousarray(root_w).astype(BF16),
        "bias": np.ascontiguousarray(rgcn_b.reshape(C, 1)),
        "lin": np.ascontiguousarray(lin_w.reshape(C, 1)).astype(BF16),
        "iota": np.broadcast_to(
            np.arange(P, dtype=np.float32), (P, P)
        ).astype(BF16).copy(),
        "gidx": gidx,
        "dstloc": dstloc,
        "alpha": alpha,
    }


def kernel(node_features, edge_index, edge_type, rel_weight, root_weight,
           rgcn_bias, lin_weight, lin_bias, **_ignored):
    node_features = np.asarray(node_features, np.float32)
    edge_index = np.asarray(edge_index)
    edge_type = np.asarray(edge_type)
    rel_weight = np.asarray(rel_weight, np.float32)
    root_weight = np.asarray(root_weight, np.float32)
    rgcn_bias = np.asarray(rgcn_bias, np.float32)
    lin_weight = np.asarray(lin_weight, np.float32)
    lin_bias = np.asarray(lin_bias, np.float32)

    cap = DEF_CAP
    while True:
        try:
            in_maps = [
                _pack_core_inputs(
                    node_features[b], edge_index[b], edge_type[b], rel_weight,
                    root_weight, rgcn_bias, lin_weight, lin_bias, cap,
                )
                for b in range(B)
            ]
            break
        except OverflowError as e:
            cap = ((int(e.args[0]) + P - 1) // P + 1) * P

    if cap not in _prog_cache:
        _prog_cache[cap] = build_program(cap)
    nc = _prog_cache[cap]

    res = run_bass_kernel_spmd(nc, in_maps, core_ids=list(range(B)))
    out = np.stack(
        [res.results[b]["scores"].reshape(N).astype(np.float32) for b in range(B)]
    )
    return (out + np.float32(lin_bias.reshape(-1)[0])).astype(np.float32)


def kernel_profiled(node_features, edge_index, edge_type, rel_weight,
                    root_weight, rgcn_bias, lin_weight, lin_bias, **_ignored):
    """Run once with NTFF tracing; returns exec_time_ns (or None)."""
    import tempfile

    in_maps = [
        _pack_core_inputs(
            np.asarray(node_features, np.float32)[b], np.asarray(edge_index)[b],
            np.asarray(edge_type)[b], np.asarray(rel_weight, np.float32),
            np.asarray(root_weight, np.float32), np.asarray(rgcn_bias, np.float32),
            np.asarray(lin_weight, np.float32), np.asarray(lin_bias, np.float32),
            DEF_CAP,
        )
        for b in range(B)
    ]
    if DEF_CAP not in _prog_cache:
        _prog_cache[DEF_CAP] = build_program(DEF_CAP)
    nc = _prog_cache[DEF_CAP]
    tmpdir = tempfile.mkdtemp(prefix="rgcn_prof_")
    res = run_bass_kernel_spmd(
        nc, in_maps, core_ids=list(range(B)), trace=True, tmpdir=tmpdir
    )
    print(f"profile artifacts in {tmpdir}")
    return res.exec_time_ns
